# revision 11
# baseline (speedup 1.0000x reference)
"""Trainium2 Bass kernel for nn_AttLayer_9972914061697 (sparse_attention).

Reference computation (jax):
    q, k, v = split(x, 3, axis=-1)              # x: [B=4, T=4096, 3C=384]
    score   = einsum('btc,bsc->bts', k, q) / sqrt(C)
    out     = softmax(score, -1) @ v            # [B, T, C=128]

Sharding: 8 cores = 4 batches x 2 T-halves (data parallel, zero comm).
Each core holds the full q, v of its batch plus its 2048-row k chunk and
produces its 2048-row output chunk. q/k/v are shipped as bf16 (the matmul
compute dtype; identical numerics to an on-device cast), output is f32.

Per-core algorithm (matmuls bf16, accumulation f32):
  - q, k land transposed in SBUF via XBAR DMA-transpose ([C, T] layout)
  - S_T[s, t] = sum_c q[s,c] k[t,c] computed as qT_chunk.T @ kT (PSUM f32)
  - P_T = exp(S_T / sqrt(C)) via ScalarE, written bf16 to SBUF
  - out[t, 0:128] & rowsum[t] in one PSUM accumulation: P_T_chunk.T @ [v | 1]
    (ones column appended to v makes the softmax denominator an extra column)
  - out = out * 1/rowsum (VectorE reciprocal + per-partition scalar mul)

The s axis runs in groups of GSC 128-row chunks, software-pipelined:
group g's QK+exp is emitted before group g-1's PV so the ScalarE exp stream
never starves, and loads for later groups are issued alongside.
"""

import numpy as np
import ml_dtypes

import concourse.bass as bass
import concourse.tile as tile
from concourse import bacc, mybir
from concourse.bass_utils import run_bass_kernel_spmd

F32 = mybir.dt.float32
BF16 = mybir.dt.bfloat16

B = 4
T = 4096
C = 128
N_CORES = 8
TL = T // 2          # 2048 t-rows per core
NSC = T // 128       # 32 s-chunks
NTT = TL // 128      # 16 t-tiles
GSC = 4              # s-chunks per group (PV flush granularity)
NG = NSC // GSC      # 8 groups
GW = GSC * 128       # s-rows per group (512)
SCALE = 1.0 / float(np.sqrt(C))


def build_nc():
    nc = bacc.Bacc()
    q = nc.declare_dram_parameter("q", [T, C], BF16, isOutput=False)
    k = nc.declare_dram_parameter("k", [TL, C], BF16, isOutput=False)
    v = nc.declare_dram_parameter("v", [T, C], BF16, isOutput=False)
    out = nc.declare_dram_parameter("out", [TL, C], F32, isOutput=True)

    vw = v[:].rearrange("(n p) c -> p n c", p=128)    # [128, 32, 128]
    ov = out[:].rearrange("(n p) c -> p n c", p=128)  # [128, 16, 128]

    with tile.TileContext(nc) as tc:
        with (
            tc.tile_pool(name="const", bufs=1) as const_pool,
            tc.tile_pool(name="qkt", bufs=1) as qkt_pool,
            tc.tile_pool(name="vbuf", bufs=1) as v_pool,
            tc.tile_pool(name="pT", bufs=3) as pT_pool,
            tc.tile_pool(name="oacc", bufs=1) as oacc_pool,
            tc.tile_pool(name="ost", bufs=2) as ost_pool,
            tc.tile_pool(name="spsum", bufs=3, space="PSUM") as spsum,
            tc.tile_pool(name="opsum", bufs=2, space="PSUM") as opsum,
        ):
            qT = qkt_pool.tile([128, T], BF16, tag="qT")     # q transposed [c, s]
            kT = qkt_pool.tile([128, TL], BF16, tag="kT")    # k transposed [c, t]
            vv = v_pool.tile([128, NSC * (C + 1)], BF16)     # 32 x [128, 129] chunks
            vv3 = vv[:].rearrange("p (n c) -> p n c", c=C + 1)
            oacc = oacc_pool.tile([128, NTT * (C + 1)], F32)
            oacc3 = oacc[:].rearrange("p (n c) -> p n c", c=C + 1)
            rtile = const_pool.tile([128, NTT], F32, tag="recip")

            # ones column of every v chunk
            nc.vector.memset(vv3[:, :, C : C + 1], 1.0)

            # warm up the ACT exp table early so the ~2.7us table load
            # overlaps the prologue DMA instead of stalling the first score
            warm = const_pool.tile([128, 8], F32, tag="warm")
            nc.vector.memset(warm[:], 0.0)
            nc.scalar.activation(
                warm[:], warm[:], mybir.ActivationFunctionType.Exp, scale=1.0
            )

            def load_qT(g, eng=None):
                (eng or nc.sync).dma_start(
                    out=qT[:, g * GW : (g + 1) * GW],
                    in_=q[g * GW : (g + 1) * GW, :],
                    transpose=True,
                )

            def load_v(g, eng=None):
                (eng or nc.sync).dma_start(
                    out=vv3[:, g * GSC : (g + 1) * GSC, 0:C],
                    in_=vw[:, g * GSC : (g + 1) * GSC, :],
                )

            # prologue loads: all XBAR transposes must stay on one engine
            # (concurrent transposes from two HWDGEs corrupt the shared XBAR);
            # q0 goes first so the first QK matmul's inputs land early, and
            # the plain (non-transpose) v load rides the Scalar HWDGE
            load_qT(0)
            for j in range(4):
                nc.sync.dma_start(
                    out=kT[:, j * 512 : (j + 1) * 512],
                    in_=k[j * 512 : (j + 1) * 512, :],
                    transpose=True,
                )
            load_qT(1)
            load_v(0, nc.scalar)

            def qk_exp_group(g, pT):
                for lc in range(GSC):
                    sc = g * GSC + lc
                    lhs = qT[:, sc * 128 : (sc + 1) * 128]
                    for h in range(2):  # two [128, 1024] halves of t
                        ps = spsum.tile([128, 1024], F32, tag="s")
                        for n in range(2):
                            t_off = h * 1024 + n * 512
                            nc.tensor.matmul(
                                ps[:, n * 512 : (n + 1) * 512],
                                lhs,
                                kT[:, t_off : t_off + 512],
                                start=True,
                                stop=True,
                            )
                        nc.scalar.activation(
                            pT[:, lc * TL + h * 1024 : lc * TL + (h + 1) * 1024],
                            ps[:],
                            mybir.ActivationFunctionType.Exp,
                            scale=SCALE,
                        )

            def pv_group(g, pT, final):
                ost = None
                for tt2 in range(NTT // 2):  # pairs of t-tiles per PSUM bank
                    op = opsum.tile([128, 2 * (C + 1)], F32, tag="o")
                    for half in range(2):
                        tt = tt2 * 2 + half
                        for lc in range(GSC):
                            nc.tensor.matmul(
                                op[:, half * (C + 1) : (half + 1) * (C + 1)],
                                pT[:, lc * TL + tt * 128 : lc * TL + (tt + 1) * 128],
                                vv3[:, g * GSC + lc, :],
                                start=(lc == 0),
                                stop=(lc == GSC - 1),
                            )
                    dst = oacc[:, tt2 * 2 * (C + 1) : (tt2 + 1) * 2 * (C + 1)]
                    if g == 0:
                        nc.vector.tensor_copy(dst, op[:])
                    else:
                        nc.vector.tensor_add(dst, dst, op[:])
                    if final:
                        # normalize + store as soon as each t-tile pair is done
                        if tt2 % 2 == 0:
                            ost = ost_pool.tile([128, 4, 128], F32, tag="ost")
                        for half in range(2):
                            tt = tt2 * 2 + half
                            nc.vector.reciprocal(
                                rtile[:, tt : tt + 1], oacc3[:, tt, C : C + 1]
                            )
                            nc.vector.tensor_scalar_mul(
                                ost[:, (tt2 % 2) * 2 + half, :],
                                oacc3[:, tt, 0:C],
                                rtile[:, tt : tt + 1],
                            )
                        if tt2 % 2 == 1:
                            tt0 = (tt2 - 1) * 2
                            nc.sync.dma_start(
                                out=ov[:, tt0 : tt0 + 4, :], in_=ost[:]
                            )

            # ---- software-pipelined main loop ----
            pT_tiles = {}
            for g in range(NG):
                if g + 2 < NG:
                    load_qT(g + 2)
                if g + 1 < NG:
                    load_v(g + 1)
                pT_g = pT_pool.tile([128, GSC * TL], BF16, tag="pT")
                pT_tiles[g] = pT_g
                qk_exp_group(g, pT_tiles[g])
                if g >= 1:
                    pv_group(g - 1, pT_tiles[g - 1], final=False)
                    del pT_tiles[g - 1]
            pv_group(NG - 1, pT_tiles[NG - 1], final=True)

    nc.finalize()
    return nc


_NC_CACHE = None


def make_in_maps(x: np.ndarray):
    xb = np.asarray(x, dtype=np.float32).astype(ml_dtypes.bfloat16)
    in_maps = []
    for core in range(N_CORES):
        b, th = core // 2, core % 2
        in_maps.append(
            {
                "q": np.ascontiguousarray(xb[b, :, 0:C]),
                "k": np.ascontiguousarray(xb[b, th * TL : (th + 1) * TL, C : 2 * C]),
                "v": np.ascontiguousarray(xb[b, :, 2 * C : 3 * C]),
            }
        )
    return in_maps


def kernel(x: np.ndarray) -> np.ndarray:
    global _NC_CACHE
    x = np.asarray(x, dtype=np.float32)
    assert x.shape == (B, T, 3 * C), x.shape

    if _NC_CACHE is None:
        _NC_CACHE = build_nc()
    nc = _NC_CACHE

    res = run_bass_kernel_spmd(nc, make_in_maps(x), core_ids=list(range(N_CORES)))

    out = np.empty((B, T, C), dtype=np.float32)
    for core in range(N_CORES):
        b, th = core // 2, core % 2
        out[b, th * TL : (th + 1) * TL] = res.results[core]["out"]
    return out


# revision 12
# speedup vs baseline: 1.1812x; 1.1812x over previous
"""Trainium2 Bass kernel for nn_AttLayer_9972914061697 (sparse_attention).

Reference computation (jax):
    q, k, v = split(x, 3, axis=-1)              # x: [B=4, T=4096, 3C=384]
    score   = einsum('btc,bsc->bts', k, q) / sqrt(C)
    out     = softmax(score, -1) @ v            # [B, T, C=128]

Sharding: 8 cores = 4 batches x 2 T-halves (data parallel, zero comm).
Each core holds the full q, v of its batch plus its 2048-row k chunk and
produces its 2048-row output chunk. q/k/v are shipped as bf16 (the matmul
compute dtype; identical numerics to an on-device cast), output is f32.

Per-core algorithm (matmuls bf16, accumulation f32):
  - q, k land transposed in SBUF via XBAR DMA-transpose ([C, T] layout)
  - S_T[s, t] = sum_c q[s,c] k[t,c] computed as qT_chunk.T @ kT (PSUM f32)
  - P_T = exp(S_T / sqrt(C)) via ScalarE, written bf16 to SBUF
  - out[t, 0:128] & rowsum[t] in one PSUM accumulation: P_T_chunk.T @ [v | 1]
    (ones column appended to v makes the softmax denominator an extra column)
  - out = out * 1/rowsum (VectorE reciprocal + per-partition scalar mul)

The s axis runs in groups of GSC 128-row chunks, software-pipelined:
group g's QK+exp is emitted before group g-1's PV so the ScalarE exp stream
never starves, and loads for later groups are issued alongside.
"""

import numpy as np
import ml_dtypes

import concourse.bass as bass
import concourse.tile as tile
from concourse import bacc, mybir
from concourse.bass_utils import run_bass_kernel_spmd

F32 = mybir.dt.float32
BF16 = mybir.dt.bfloat16

B = 4
T = 4096
C = 128
N_CORES = 8
TL = T // 2          # 2048 t-rows per core
NSC = T // 128       # 32 s-chunks
NTT = TL // 128      # 16 t-tiles
GSC = 4              # s-chunks per group (PV flush granularity)
NG = NSC // GSC      # 8 groups
GW = GSC * 128       # s-rows per group (512)
SCALE = 1.0 / float(np.sqrt(C))


def build_nc():
    nc = bacc.Bacc()
    q = nc.declare_dram_parameter("q", [T, C], BF16, isOutput=False)
    k = nc.declare_dram_parameter("k", [TL, C], BF16, isOutput=False)
    v = nc.declare_dram_parameter("v", [T, C], BF16, isOutput=False)
    out = nc.declare_dram_parameter("out", [TL, C], F32, isOutput=True)

    vw = v[:].rearrange("(n p) c -> p n c", p=128)    # [128, 32, 128]
    ov = out[:].rearrange("(n p) c -> p n c", p=128)  # [128, 16, 128]

    with tile.TileContext(nc) as tc:
        with (
            tc.tile_pool(name="const", bufs=1) as const_pool,
            tc.tile_pool(name="qkt", bufs=1) as qkt_pool,
            tc.tile_pool(name="vbuf", bufs=1) as v_pool,
            tc.tile_pool(name="pT", bufs=3) as pT_pool,
            tc.tile_pool(name="oacc", bufs=1) as oacc_pool,
            tc.tile_pool(name="ost", bufs=2) as ost_pool,
            tc.tile_pool(name="spsum", bufs=3, space="PSUM") as spsum,
            tc.tile_pool(name="opsum", bufs=2, space="PSUM") as opsum,
        ):
            qT = qkt_pool.tile([128, T], BF16, tag="qT")     # q transposed [c, s]
            kT = qkt_pool.tile([128, TL], BF16, tag="kT")    # k transposed [c, t]
            vv = v_pool.tile([128, NSC * (C + 1)], BF16)     # 32 x [128, 129] chunks
            vv3 = vv[:].rearrange("p (n c) -> p n c", c=C + 1)
            oacc = oacc_pool.tile([128, NTT * (C + 1)], F32)
            oacc3 = oacc[:].rearrange("p (n c) -> p n c", c=C + 1)
            rtile = const_pool.tile([128, NTT], F32, tag="recip")

            # ones column of every v chunk
            nc.vector.memset(vv3[:, :, C : C + 1], 1.0)

            # warm up the ACT exp table early so the ~2.7us table load
            # overlaps the prologue DMA instead of stalling the first score
            warm = const_pool.tile([128, 8], F32, tag="warm")
            nc.vector.memset(warm[:], 0.0)
            nc.scalar.activation(
                warm[:], warm[:], mybir.ActivationFunctionType.Exp, scale=1.0
            )

            def load_qT(g, eng=None):
                (eng or nc.sync).dma_start(
                    out=qT[:, g * GW : (g + 1) * GW],
                    in_=q[g * GW : (g + 1) * GW, :],
                    transpose=True,
                )

            def load_v(g, eng=None):
                (eng or nc.sync).dma_start(
                    out=vv3[:, g * GSC : (g + 1) * GSC, 0:C],
                    in_=vw[:, g * GSC : (g + 1) * GSC, :],
                )

            # prologue loads: all DMA stays on the Sync HWDGE — concurrent
            # XBAR transposes from two HWDGEs corrupt the shared XBAR, and
            # mixing plain DMA on the other HWDGE serializes on mode switches
            for j in range(4):
                nc.sync.dma_start(
                    out=kT[:, j * 512 : (j + 1) * 512],
                    in_=k[j * 512 : (j + 1) * 512, :],
                    transpose=True,
                )
            load_qT(0)
            load_qT(1)
            load_v(0)

            def qk_exp_group(g, pT):
                for lc in range(GSC):
                    sc = g * GSC + lc
                    lhs = qT[:, sc * 128 : (sc + 1) * 128]
                    for h in range(2):  # two [128, 1024] halves of t
                        ps = spsum.tile([128, 1024], F32, tag="s")
                        for n in range(2):
                            t_off = h * 1024 + n * 512
                            nc.tensor.matmul(
                                ps[:, n * 512 : (n + 1) * 512],
                                lhs,
                                kT[:, t_off : t_off + 512],
                                start=True,
                                stop=True,
                            )
                        nc.scalar.activation(
                            pT[:, lc * TL + h * 1024 : lc * TL + (h + 1) * 1024],
                            ps[:],
                            mybir.ActivationFunctionType.Exp,
                            scale=SCALE,
                        )

            def pv_group(g, pT, final):
                ost = None
                for tt2 in range(NTT // 2):  # pairs of t-tiles per PSUM bank
                    op = opsum.tile([128, 2 * (C + 1)], F32, tag="o")
                    for half in range(2):
                        tt = tt2 * 2 + half
                        for lc in range(GSC):
                            nc.tensor.matmul(
                                op[:, half * (C + 1) : (half + 1) * (C + 1)],
                                pT[:, lc * TL + tt * 128 : lc * TL + (tt + 1) * 128],
                                vv3[:, g * GSC + lc, :],
                                start=(lc == 0),
                                stop=(lc == GSC - 1),
                            )
                    dst = oacc[:, tt2 * 2 * (C + 1) : (tt2 + 1) * 2 * (C + 1)]
                    if g == 0:
                        nc.vector.tensor_copy(dst, op[:])
                    else:
                        nc.vector.tensor_add(dst, dst, op[:])
                    if final:
                        # normalize + store as soon as each t-tile pair is done
                        if tt2 % 2 == 0:
                            ost = ost_pool.tile([128, 4, 128], F32, tag="ost")
                        for half in range(2):
                            tt = tt2 * 2 + half
                            nc.vector.reciprocal(
                                rtile[:, tt : tt + 1], oacc3[:, tt, C : C + 1]
                            )
                            nc.vector.tensor_scalar_mul(
                                ost[:, (tt2 % 2) * 2 + half, :],
                                oacc3[:, tt, 0:C],
                                rtile[:, tt : tt + 1],
                            )
                        if tt2 % 2 == 1:
                            tt0 = (tt2 - 1) * 2
                            nc.sync.dma_start(
                                out=ov[:, tt0 : tt0 + 4, :], in_=ost[:]
                            )

            # ---- software-pipelined main loop ----
            pT_tiles = {}
            for g in range(NG):
                if g + 2 < NG:
                    load_qT(g + 2)
                if g + 1 < NG:
                    load_v(g + 1)
                pT_g = pT_pool.tile([128, GSC * TL], BF16, tag="pT")
                pT_tiles[g] = pT_g
                qk_exp_group(g, pT_tiles[g])
                if g >= 1:
                    pv_group(g - 1, pT_tiles[g - 1], final=False)
                    del pT_tiles[g - 1]
            pv_group(NG - 1, pT_tiles[NG - 1], final=True)

    nc.finalize()
    return nc


_NC_CACHE = None


def make_in_maps(x: np.ndarray):
    xb = np.asarray(x, dtype=np.float32).astype(ml_dtypes.bfloat16)
    in_maps = []
    for core in range(N_CORES):
        b, th = core // 2, core % 2
        in_maps.append(
            {
                "q": np.ascontiguousarray(xb[b, :, 0:C]),
                "k": np.ascontiguousarray(xb[b, th * TL : (th + 1) * TL, C : 2 * C]),
                "v": np.ascontiguousarray(xb[b, :, 2 * C : 3 * C]),
            }
        )
    return in_maps


def kernel(x: np.ndarray) -> np.ndarray:
    global _NC_CACHE
    x = np.asarray(x, dtype=np.float32)
    assert x.shape == (B, T, 3 * C), x.shape

    if _NC_CACHE is None:
        _NC_CACHE = build_nc()
    nc = _NC_CACHE

    res = run_bass_kernel_spmd(nc, make_in_maps(x), core_ids=list(range(N_CORES)))

    out = np.empty((B, T, C), dtype=np.float32)
    for core in range(N_CORES):
        b, th = core // 2, core % 2
        out[b, th * TL : (th + 1) * TL] = res.results[core]["out"]
    return out


# revision 13
# speedup vs baseline: 1.1837x; 1.0021x over previous
"""Trainium2 Bass kernel for nn_AttLayer_9972914061697 (sparse_attention).

Reference computation (jax):
    q, k, v = split(x, 3, axis=-1)              # x: [B=4, T=4096, 3C=384]
    score   = einsum('btc,bsc->bts', k, q) / sqrt(C)
    out     = softmax(score, -1) @ v            # [B, T, C=128]

Sharding: 8 cores = 4 batches x 2 T-halves (data parallel, zero comm).
Each core holds the full q, v of its batch plus its 2048-row k chunk and
produces its 2048-row output chunk. q/k/v are shipped as bf16 (the matmul
compute dtype; identical numerics to an on-device cast), output is f32.

Per-core algorithm (matmuls bf16, accumulation f32):
  - q, k land transposed in SBUF via XBAR DMA-transpose ([C, T] layout)
  - S_T[s, t] = sum_c q[s,c] k[t,c] computed as qT_chunk.T @ kT (PSUM f32)
  - P_T = exp(S_T / sqrt(C)) via ScalarE, written bf16 to SBUF
  - out[t, 0:128] & rowsum[t] in one PSUM accumulation: P_T_chunk.T @ [v | 1]
    (ones column appended to v makes the softmax denominator an extra column)
  - out = out * 1/rowsum (VectorE reciprocal + per-partition scalar mul)

The s axis runs in groups of GSC 128-row chunks, software-pipelined:
group g's QK+exp is emitted before group g-1's PV so the ScalarE exp stream
never starves, and loads for later groups are issued alongside.
"""

import numpy as np
import ml_dtypes

import concourse.bass as bass
import concourse.tile as tile
from concourse import bacc, mybir
from concourse.bass_utils import run_bass_kernel_spmd

F32 = mybir.dt.float32
BF16 = mybir.dt.bfloat16

B = 4
T = 4096
C = 128
N_CORES = 8
TL = T // 2          # 2048 t-rows per core
NSC = T // 128       # 32 s-chunks
NTT = TL // 128      # 16 t-tiles
GSC = 4              # s-chunks per group (PV flush granularity)
NG = NSC // GSC      # 8 groups
GW = GSC * 128       # s-rows per group (512)
SCALE = 1.0 / float(np.sqrt(C))


def build_nc():
    nc = bacc.Bacc()
    q = nc.declare_dram_parameter("q", [T, C], BF16, isOutput=False)
    k = nc.declare_dram_parameter("k", [TL, C], BF16, isOutput=False)
    v = nc.declare_dram_parameter("v", [T, C], BF16, isOutput=False)
    out = nc.declare_dram_parameter("out", [TL, C], F32, isOutput=True)

    vw = v[:].rearrange("(n p) c -> p n c", p=128)    # [128, 32, 128]
    ov = out[:].rearrange("(n p) c -> p n c", p=128)  # [128, 16, 128]

    with tile.TileContext(nc) as tc:
        with (
            tc.tile_pool(name="const", bufs=1) as const_pool,
            tc.tile_pool(name="qkt", bufs=1) as qkt_pool,
            tc.tile_pool(name="vbuf", bufs=1) as v_pool,
            tc.tile_pool(name="pT", bufs=3) as pT_pool,
            tc.tile_pool(name="oacc", bufs=1) as oacc_pool,
            tc.tile_pool(name="ost", bufs=2) as ost_pool,
            tc.tile_pool(name="spsum", bufs=3, space="PSUM") as spsum,
            tc.tile_pool(name="opsum", bufs=2, space="PSUM") as opsum,
        ):
            qT = qkt_pool.tile([128, T], BF16, tag="qT")     # q transposed [c, s]
            kT = qkt_pool.tile([128, TL], BF16, tag="kT")    # k transposed [c, t]
            vv = v_pool.tile([128, NSC * (C + 1)], BF16)     # 32 x [128, 129] chunks
            vv3 = vv[:].rearrange("p (n c) -> p n c", c=C + 1)
            oacc = oacc_pool.tile([128, NTT * (C + 1)], F32)
            oacc3 = oacc[:].rearrange("p (n c) -> p n c", c=C + 1)
            rtile = const_pool.tile([128, NTT], F32, tag="recip")

            # ones column of every v chunk
            nc.vector.memset(vv3[:, :, C : C + 1], 1.0)

            # warm up the ACT exp table early so the ~2.7us table load
            # overlaps the prologue DMA instead of stalling the first score
            warm = const_pool.tile([128, 8], F32, tag="warm")
            nc.vector.memset(warm[:], 0.0)
            nc.scalar.activation(
                warm[:], warm[:], mybir.ActivationFunctionType.Exp, scale=1.0
            )

            def load_qT(g, eng=None):
                (eng or nc.sync).dma_start(
                    out=qT[:, g * GW : (g + 1) * GW],
                    in_=q[g * GW : (g + 1) * GW, :],
                    transpose=True,
                )

            def load_v(g, eng=None):
                (eng or nc.sync).dma_start(
                    out=vv3[:, g * GSC : (g + 1) * GSC, 0:C],
                    in_=vw[:, g * GSC : (g + 1) * GSC, :],
                )

            # prologue loads: all DMA stays on the Sync HWDGE — concurrent
            # XBAR transposes from two HWDGEs corrupt the shared XBAR, and
            # mixing plain DMA on the other HWDGE serializes on mode switches
            load_qT(0)
            for j in range(4):
                nc.sync.dma_start(
                    out=kT[:, j * 512 : (j + 1) * 512],
                    in_=k[j * 512 : (j + 1) * 512, :],
                    transpose=True,
                )
            load_qT(1)
            load_v(0)

            def qk_exp_group(g, pT):
                for lc in range(GSC):
                    sc = g * GSC + lc
                    lhs = qT[:, sc * 128 : (sc + 1) * 128]
                    for h in range(2):  # two [128, 1024] halves of t
                        ps = spsum.tile([128, 1024], F32, tag="s")
                        for n in range(2):
                            t_off = h * 1024 + n * 512
                            nc.tensor.matmul(
                                ps[:, n * 512 : (n + 1) * 512],
                                lhs,
                                kT[:, t_off : t_off + 512],
                                start=True,
                                stop=True,
                            )
                        nc.scalar.activation(
                            pT[:, lc * TL + h * 1024 : lc * TL + (h + 1) * 1024],
                            ps[:],
                            mybir.ActivationFunctionType.Exp,
                            scale=SCALE,
                        )

            def pv_group(g, pT, final):
                ost = None
                for tt2 in range(NTT // 2):  # pairs of t-tiles per PSUM bank
                    op = opsum.tile([128, 2 * (C + 1)], F32, tag="o")
                    for half in range(2):
                        tt = tt2 * 2 + half
                        for lc in range(GSC):
                            nc.tensor.matmul(
                                op[:, half * (C + 1) : (half + 1) * (C + 1)],
                                pT[:, lc * TL + tt * 128 : lc * TL + (tt + 1) * 128],
                                vv3[:, g * GSC + lc, :],
                                start=(lc == 0),
                                stop=(lc == GSC - 1),
                            )
                    dst = oacc[:, tt2 * 2 * (C + 1) : (tt2 + 1) * 2 * (C + 1)]
                    if g == 0:
                        nc.vector.tensor_copy(dst, op[:])
                    else:
                        nc.vector.tensor_add(dst, dst, op[:])
                    if final:
                        # normalize + store as soon as each t-tile pair is done
                        if tt2 % 2 == 0:
                            ost = ost_pool.tile([128, 4, 128], F32, tag="ost")
                        for half in range(2):
                            tt = tt2 * 2 + half
                            nc.vector.reciprocal(
                                rtile[:, tt : tt + 1], oacc3[:, tt, C : C + 1]
                            )
                            nc.vector.tensor_scalar_mul(
                                ost[:, (tt2 % 2) * 2 + half, :],
                                oacc3[:, tt, 0:C],
                                rtile[:, tt : tt + 1],
                            )
                        if tt2 % 2 == 1:
                            tt0 = (tt2 - 1) * 2
                            nc.sync.dma_start(
                                out=ov[:, tt0 : tt0 + 4, :], in_=ost[:]
                            )

            # ---- software-pipelined main loop ----
            pT_tiles = {}
            for g in range(NG):
                if g + 2 < NG:
                    load_qT(g + 2)
                if g + 1 < NG:
                    load_v(g + 1)
                pT_g = pT_pool.tile([128, GSC * TL], BF16, tag="pT")
                pT_tiles[g] = pT_g
                qk_exp_group(g, pT_tiles[g])
                if g >= 1:
                    pv_group(g - 1, pT_tiles[g - 1], final=False)
                    del pT_tiles[g - 1]
            pv_group(NG - 1, pT_tiles[NG - 1], final=True)

    nc.finalize()
    return nc


_NC_CACHE = None


def make_in_maps(x: np.ndarray):
    xb = np.asarray(x, dtype=np.float32).astype(ml_dtypes.bfloat16)
    in_maps = []
    for core in range(N_CORES):
        b, th = core // 2, core % 2
        in_maps.append(
            {
                "q": np.ascontiguousarray(xb[b, :, 0:C]),
                "k": np.ascontiguousarray(xb[b, th * TL : (th + 1) * TL, C : 2 * C]),
                "v": np.ascontiguousarray(xb[b, :, 2 * C : 3 * C]),
            }
        )
    return in_maps


def kernel(x: np.ndarray) -> np.ndarray:
    global _NC_CACHE
    x = np.asarray(x, dtype=np.float32)
    assert x.shape == (B, T, 3 * C), x.shape

    if _NC_CACHE is None:
        _NC_CACHE = build_nc()
    nc = _NC_CACHE

    res = run_bass_kernel_spmd(nc, make_in_maps(x), core_ids=list(range(N_CORES)))

    out = np.empty((B, T, C), dtype=np.float32)
    for core in range(N_CORES):
        b, th = core // 2, core % 2
        out[b, th * TL : (th + 1) * TL] = res.results[core]["out"]
    return out
